# revision 3
# baseline (speedup 1.0000x reference)
"""Masked-BCE valid-region loss on 8 Trainium2 NeuronCores — bf16.

Host casts the three big tensors to bf16 and packs each image as one
flat [128, 6144] row built from per-chunk [p|n|x] segments, so every
chunk DMA is a single contiguous 1.5-6KB descriptor per partition and
pays the ~2us completion-sem lag exactly once.

Engine split (measured on HW):
  DVE : pt = min(p,n) [2x] ; mask = (pt>0.5) [4x] ; xm = mask*x [2x]
  PE  : count = ones^T @ mask ; per-image sxm = ones^T @ xm  (PSUM acc)
  ACT : e = exp(xm) per EXP-group ; sp = ln(e+1) per LN-group with
        fused accum -> ssp (grouping cuts per-op overhead and
        READ_ACCUMULATOR count on the bottleneck engine)
  DVE : PSUM -> SBUF copies of count/sxm (overlap the final ACT passes)

Chunk schedule [512,512,1024 | 1024,1024 | 1024,1024 | 1024,512,512]:
a small single-sem first chunk starts ACT early; mid-stream EXPs are
pair-merged (the ACT queue runs deep there so op-count is pure win);
the last image tapers so the post-stream serial tail is short. The ssp
output ships in two pieces; only a 2-column slice rides the tail.

Host combine (y constant per image):
    bce = softplus(x) - x*y
    sum(bce*m) = sum_masked softplus(x) - y * sum(x*m)
    softplus(x*m) = softplus(x) where m==1, ln(2) where m==0
 => sum_masked softplus(x) = sum softplus(x*m) - (N - count)*ln(2)
"""

import sys

for _p in ("/opt/trn_rl_repo", "/root/.axon_site/_ro/trn_rl_repo"):
    if _p not in sys.path:
        sys.path.append(_p)

import ml_dtypes
import numpy as np

import concourse.bacc as bacc
import concourse.tile as tile
from concourse import mybir
from concourse.bass_utils import run_bass_kernel_spmd

B, H, W = 32, 512, 512
N_CORES = 8
IMGS_PER_CORE = B // N_CORES  # 4
P = 128
FD = (H * W) // P  # 2048
N_PER_IMG = H * W  # 262144
MM = 512  # matmul moving width == PSUM row width

# Per-image chunk segmentation (cols); flat [p|n|x] per chunk in DRAM.
IMG_SEGS = {
    0: [512, 512, 1024],
    1: [1024, 1024],
    2: [1024, 1024],
    3: [1024, 512, 512],
}
# CHUNKS: (img, col0, cols, flat_off)
CHUNKS = []
for _i in range(IMGS_PER_CORE):
    _c0 = 0
    _off = 0
    for _cols in IMG_SEGS[_i]:
        CHUNKS.append((_i, _c0, _cols, _off))
        _c0 += _cols
        _off += 3 * _cols
# EXP ops read per-group xm tiles; mid-stream pairs are merged.
EXP_GROUPS = [[0], [1], [2], [3, 4], [5, 6], [7, 8], [9]]
# LN ops cover LN groups (a prefix-union of EXP groups).
LN_GROUPS = [[0, 1, 2], [3, 4], [5, 6], [7, 8], [9]]
N_COLS = len(LN_GROUPS)
SSP_SPLIT = 3  # ssp cols [0:3] ship mid-kernel; the rest ride the tail

_nc_cache = None


def _patch_act_tables():
    """Steer every activation to `natural_log_exp_and_others` so bacc
    emits a single ACT_TABLE_LOAD (exp and ln share that set)."""
    import concourse.hw_specs as hw_specs

    if getattr(bacc, "_act_tables_patched", False):
        return
    orig = hw_specs.get_activation_tables

    def patched(module_arch):
        tables = orig(module_arch)
        keep = "natural_log_exp_and_others"
        if keep in tables:
            tables = {
                name: (funcs if name == keep else set())
                for name, funcs in tables.items()
            }
        return tables

    bacc.get_activation_tables = patched
    bacc._act_tables_patched = True


def _groups_meta(groups, chunks):
    gi_of = {}
    off_of = {}
    for gi, grp in enumerate(groups):
        off = 0
        for ci in grp:
            gi_of[ci] = gi
            off_of[ci] = off
            off += chunks[ci][2]
    return gi_of, off_of


def _build_bass():
    _patch_act_tables()
    f32 = mybir.dt.float32
    bf16 = mybir.dt.bfloat16
    nc = bacc.Bacc()
    xpn_d = nc.dram_tensor(
        "xpn", [IMGS_PER_CORE, P, 3 * FD], bf16, kind="ExternalInput"
    )
    ssp_d = nc.dram_tensor("ssp", [P, N_COLS], f32, kind="ExternalOutput")
    # red[0, 0:2048] = per-image sum(x*m) (512-wide regions); [2048:2560] = count.
    red_d = nc.dram_tensor("red", [1, 5 * MM], f32, kind="ExternalOutput")

    exp_gi, exp_off = _groups_meta(EXP_GROUPS, CHUNKS)
    ln_gi, ln_off = _groups_meta(LN_GROUPS, CHUNKS)

    with tile.TileContext(nc) as tc:
        with (
            tc.tile_pool(name="io", bufs=1) as io_pool,
            tc.tile_pool(name="stats", bufs=1) as stats_pool,
            tc.tile_pool(name="psum", bufs=1, space="PSUM") as psum_pool,
        ):
            ssp_t = stats_pool.tile([P, N_COLS], f32)
            ones = stats_pool.tile([P, 1], bf16)
            nc.vector.memset(ones, 1.0)
            sxm_ps = psum_pool.tile([1, 4 * MM], f32)
            cnt_ps = psum_pool.tile([1, MM], f32)
            xm_g = [
                stats_pool.tile(
                    [P, sum(CHUNKS[ci][2] for ci in grp)],
                    bf16,
                    name=f"xmg{gi}",
                    tag=f"xmg{gi}",
                )
                for gi, grp in enumerate(EXP_GROUPS)
            ]
            et_g = [
                stats_pool.tile(
                    [P, sum(CHUNKS[ci][2] for ci in grp)],
                    bf16,
                    name=f"etg{gi}",
                    tag=f"etg{gi}",
                )
                for gi, grp in enumerate(LN_GROUPS)
            ]

            # One single-descriptor DMA per chunk (one sem lag each).
            chunk_tiles = []
            for ci, (i, c0, cols, off) in enumerate(CHUNKS):
                t = io_pool.tile([P, 3, cols], bf16, tag=f"c{ci}")
                nc.sync.dma_start(
                    out=t, in_=xpn_d[i][:, off : off + 3 * cols]
                )
                chunk_tiles.append(t)

            total = sum(cols // MM for _, _, cols, _ in CHUNKS)
            img_mm_total = {i: 0 for i in range(IMGS_PER_CORE)}
            for i, _, cols, _ in CHUNKS:
                img_mm_total[i] += cols // MM
            mm_done = 0
            img_mm_done = {i: 0 for i in range(IMGS_PER_CORE)}
            for ci, (i, c0, cols, off) in enumerate(CHUNKS):
                t = chunk_tiles[ci]
                pt = t[:, 0, :]
                nt = t[:, 1, :]
                tx = t[:, 2, :]
                # pt = min(p, n); bf16 tensor_tensor runs 2x.
                nc.vector.tensor_tensor(
                    out=pt, in0=pt, in1=nt, op=mybir.AluOpType.min
                )
                # mask = (pt > 0.5) in bf16 {0,1}; plain tensor_scalar is 4x.
                nc.vector.tensor_scalar(
                    out=nt,
                    in0=pt,
                    scalar1=0.5,
                    scalar2=None,
                    op0=mybir.AluOpType.is_gt,
                )
                # xm = mask * x into this chunk's slice of its EXP-group
                # tile; bf16 tensor_tensor 2x.
                egi = exp_gi[ci]
                eoff = exp_off[ci]
                xmt = xm_g[egi][:, eoff : eoff + cols]
                nc.vector.tensor_tensor(
                    out=xmt, in0=nt, in1=tx, op=mybir.AluOpType.mult
                )
                # TensorE reductions into PSUM: count over mask (one global
                # accumulation group), sum(x*m) per image (one group/image).
                for c in range(cols // MM):
                    nc.tensor.matmul(
                        cnt_ps,
                        ones,
                        nt[:, c * MM : (c + 1) * MM],
                        start=(mm_done == 0),
                        stop=(mm_done == total - 1),
                    )
                    nc.tensor.matmul(
                        sxm_ps[:, i * MM : (i + 1) * MM],
                        ones,
                        xmt[:, c * MM : (c + 1) * MM],
                        start=(img_mm_done[i] == 0),
                        stop=(img_mm_done[i] == img_mm_total[i] - 1),
                    )
                    mm_done += 1
                    img_mm_done[i] += 1
                # e = exp(xm) once the EXP group is complete, writing the
                # group's slice of its LN-group tile.
                if ci == EXP_GROUPS[egi][-1]:
                    lgi = ln_gi[ci]
                    loff = ln_off[EXP_GROUPS[egi][0]]
                    gcols = xm_g[egi].shape[1]
                    nc.scalar.activation(
                        out=et_g[lgi][:, loff : loff + gcols],
                        in_=xm_g[egi],
                        func=mybir.ActivationFunctionType.Exp,
                    )
                # One LN covers the whole LN group: sp = ln(e+1), accum.
                if ci == LN_GROUPS[ln_gi[ci]][-1]:
                    lgi = ln_gi[ci]
                    nc.scalar.activation(
                        out=et_g[lgi],
                        in_=et_g[lgi],
                        func=mybir.ActivationFunctionType.Ln,
                        bias=1.0,
                        accum_out=ssp_t[:, lgi : lgi + 1],
                    )
                    if lgi == SSP_SPLIT - 1:
                        # Early ssp columns ship mid-kernel, fully hidden.
                        nc.sync.dma_start(
                            out=ssp_d[:, 0:SSP_SPLIT],
                            in_=ssp_t[:, 0:SSP_SPLIT],
                        )

            # Export PSUM through SBUF (DMA cannot read PSUM). These DVE
            # copies depend only on the last matmuls, so they overlap the
            # final ACT passes.
            red_sb = stats_pool.tile([1, 5 * MM], f32)
            nc.vector.tensor_copy(out=red_sb[:, 0 : 4 * MM], in_=sxm_ps)
            nc.vector.tensor_copy(out=red_sb[:, 4 * MM : 5 * MM], in_=cnt_ps)
            nc.sync.dma_start(out=red_d[:], in_=red_sb)
            nc.sync.dma_start(
                out=ssp_d[:, SSP_SPLIT:], in_=ssp_t[:, SSP_SPLIT:]
            )
    nc.finalize()
    return nc


def _get_nc():
    global _nc_cache
    if _nc_cache is None:
        _nc_cache = _build_bass()
    return _nc_cache


def _make_in_maps(cancer_logits, prostate_mask, needle_mask):
    bf16 = ml_dtypes.bfloat16
    x = np.asarray(cancer_logits, dtype=np.float32).reshape(B, P, FD)
    p = np.asarray(prostate_mask, dtype=np.float32).reshape(B, P, FD)
    n = np.asarray(needle_mask, dtype=np.float32).reshape(B, P, FD)
    xb = x.astype(bf16)
    pb = p.astype(bf16)
    nb = n.astype(bf16)
    xpn = np.empty((B, P, 3 * FD), dtype=bf16)
    for b in range(B):
        i = b % IMGS_PER_CORE
        parts = []
        c0 = 0
        for cols in IMG_SEGS[i]:
            sl = slice(c0, c0 + cols)
            parts += [pb[b][:, sl], nb[b][:, sl], xb[b][:, sl]]
            c0 += cols
        xpn[b] = np.concatenate(parts, axis=1)
    return [
        {"xpn": xpn[c * IMGS_PER_CORE : (c + 1) * IMGS_PER_CORE]}
        for c in range(N_CORES)
    ]


def _combine(results, label):
    y = np.asarray(label, dtype=np.float64).reshape(B)
    ln2 = np.log(2.0)
    num = 0.0
    cnt = 0.0
    for c in range(N_CORES):
        red = np.asarray(results[c]["red"], dtype=np.float64).reshape(5 * MM)
        ssp = np.asarray(results[c]["ssp"], dtype=np.float64)
        sxm_i = red[: 4 * MM].reshape(4, MM).sum(axis=1)  # per image
        c_core = red[4 * MM :].sum()
        a_sum = ssp.sum() - (IMGS_PER_CORE * N_PER_IMG - c_core) * ln2
        y_i = y[c * IMGS_PER_CORE : (c + 1) * IMGS_PER_CORE]
        num += a_sum - (y_i * sxm_i).sum()
        cnt += c_core
    return np.float32(num / max(cnt, 1.0))


def kernel(cancer_logits, label, prostate_mask, needle_mask):
    nc = _get_nc()
    in_maps = _make_in_maps(cancer_logits, prostate_mask, needle_mask)
    res = run_bass_kernel_spmd(nc, in_maps, core_ids=list(range(N_CORES)))
    return _combine(res.results, label)
